# revision 20
# baseline (speedup 1.0000x reference)
"""ProbAttention (sparse_attention) Trainium2 Bass kernel.

Math (restructured but exact vs the reference):
  Q/K/V = proj(q/k/v); per (b,h):
    S = Q @ K^T (fp32, PE)
    Mmax[l] = max_{j in sample-set(l)} S[l,j]    (fused DVE TTR, fp8 additive mask)
    Msum[l] = Q[l] . Ksum[l], Ksum = counts @ K  (PE matmul with fp8 count mask)
    M = Mmax - Msum/L
    top-40 of M (exact, ordered) -> one-hot E
    ctx rows r<40: (Vsum + (e^{x_r}-1) V[j_r]) / (L-1+e^{x_r}),  x_r = Q[j_r].K[L-1]
    ctx rows >=40: Vsum/L  (identical rows -> computed once)
  out = ctx @ Wo + bo

Sharding: core c in 0..7 handles batch b=c//4 and heads {2*(c%4), 2*(c%4)+1}
(a 128-wide column slice of the projections). Each core returns a [41, 512]
partial (40 special rows + 1 common row) of ctx_heads @ Wo_slice; the host sums
4 partials per batch, adds bo, and broadcasts the common row to rows 40..2047.
"""

import numpy as np
import ml_dtypes

import concourse.bass as bass
import concourse.mybir as mybir
from concourse.tile import TileContext
from concourse.bass_utils import run_bass_kernel_spmd

F32 = mybir.dt.float32
F16 = mybir.dt.float16
BF16 = mybir.dt.bfloat16
FP8 = mybir.dt.float8e4
U16 = mybir.dt.uint16
U32 = mybir.dt.uint32

B, L, D, H, DK = 2, 2048, 512, 8, 64
NTOP = 40
NT = L // 128          # 16 row tiles
NEG = -3.0e38
ALU = mybir.AluOpType
ACTF = mybir.ActivationFunctionType


class _TileContext(TileContext):
    """TileContext that splits sem waits one-per-instruction (this walrus
    build rejects >1 sync wait on any instruction)."""

    def _split_waits_one_per_inst(self):
        nc = self.nc
        eng = {
            mybir.EngineType.PE: nc.tensor,
            mybir.EngineType.DVE: nc.vector,
            mybir.EngineType.Activation: nc.scalar,
            mybir.EngineType.Pool: nc.gpsimd,
            mybir.EngineType.SP: nc.sync,
        }
        cur_insts = nc.cur_bb.bb.instructions
        for bbb in list(nc.bb_map.values()):
            insts = bbb.bb.instructions
            fixups = [
                inst for inst in insts
                if inst.sync_info and inst.sync_info.on_wait
                and len(inst.sync_info.on_wait) > 1
            ]
            for inst in fixups:
                si = inst.sync_info
                waits = list(si.on_wait)
                si.on_wait = waits[-1:]
                for w in waits[:-1]:
                    nop = eng[inst.engine].nop()
                    nop.ins.sync_info = mybir.SyncInfo(on_wait=[w], on_update=[])
                    ni = next(i for i, x in enumerate(cur_insts)
                              if getattr(x, "name", None) == nop.ins.name)
                    nop_inst = cur_insts.pop(ni)
                    di = next(i for i, x in enumerate(insts)
                              if getattr(x, "name", None) == inst.name)
                    insts.insert(di, nop_inst)

    def _drain_and_barrier(self, tick_clock, wait_clock):
        from concourse.vector_clock import ScopedClock
        nc = self.nc
        self._split_waits_one_per_inst()
        drain_inst = nc.sync.drain()
        wait_clock.add_sem_waits(
            drain_inst.ins, ScopedClock({None: tick_clock.global_clock})
        )
        si = drain_inst.ins.sync_info
        waits = list(si.on_wait or [])
        if len(waits) > 1:
            si.on_wait = waits[-1:]
            bb = nc.cur_bb.bb
            insts = bb.instructions
            for w in waits[:-1]:
                nop = nc.sync.nop()
                nop.ins.sync_info = mybir.SyncInfo(on_wait=[w], on_update=[])
                di = next(
                    i for i, x in enumerate(insts)
                    if getattr(x, "name", None) == drain_inst.ins.name
                )
                ni = next(
                    i for i, x in enumerate(insts)
                    if getattr(x, "name", None) == nop.ins.name
                )
                insts.insert(di, insts.pop(ni))
        nc.all_engine_barrier()
        popped = nc._tile_sem_poison_stack.pop()
        assert popped is self._sem_poison
        nc.clear_and_free_semaphores(list(self.sems.allocated().values()))
        nc.all_engine_barrier()


def build_nc():
    nc = bass.Bass()
    dp = nc.declare_dram_parameter
    qb = dp("qb", [L, D], F32, isOutput=False)
    kb = dp("kb", [L, D], F32, isOutput=False)
    vb = dp("vb", [L, D], F32, isOutput=False)
    wq = dp("wq", [D, 128], F32, isOutput=False)
    wk = dp("wk", [D, 128], F32, isOutput=False)
    wv = dp("wv", [D, 128], F32, isOutput=False)
    wo = dp("wo", [128, D], F32, isOutput=False)
    bq = dp("bq", [128, 1], F32, isOutput=False)
    bk = dp("bk", [128, 1], F32, isOutput=False)
    bv = dp("bv", [128, 1], F32, isOutput=False)
    cmax = dp("cmax", [L, L], FP8, isOutput=False)    # l-major: 0 sel / -240
    csumT = dp("csumT", [L, L], FP8, isOutput=False)  # j-major counts
    io128 = dp("io128", [128, 1], F32, isOutput=False)  # 0..127
    tb64 = dp("tb64", [64, 1], F32, isOutput=False)     # tile base per level-1 row
    outp = dp("outp", [41, D], F32, isOutput=True)

    with _TileContext(nc) as tc:
        per = tc.alloc_tile_pool(name="per", bufs=1)

        # ---- persistent SBUF ----
        wq_t = per.tile([128, 4 * 128], F32, tag="wq")
        wk_t = per.tile([128, 4 * 128], F32, tag="wk")
        wv_t = per.tile([128, 4 * 128], F32, tag="wv")
        wo_t = per.tile([128, D], F32, tag="wo")
        bq_t = per.tile([128, 1], F32, tag="bq")
        bk_t = per.tile([128, 1], F32, tag="bk")
        bv_t = per.tile([128, 1], F32, tag="bv")
        io_t = per.tile([128, 1], F32, tag="io")
        tb_t = per.tile([64, 1], F32, tag="tb")
        qbT = [per.tile([128, L], F32, tag=f"qbT{c}", name=f"qbT{c}") for c in range(4)]
        kbT = [per.tile([128, L], F32, tag=f"kbT{c}", name=f"kbT{c}") for c in range(4)]
        vbT = [per.tile([128, L], F32, tag=f"vbT{c}", name=f"vbT{c}") for c in range(4)]
        QT = per.tile([128, L], F32, tag="QT")
        KT = per.tile([128, L], F32, tag="KT")
        VT = per.tile([128, L], F32, tag="VT")
        vqnat = per.tile([128, NT * 256], F32, tag="vqnat")  # [V_t | Q_t] per tile
        knat16 = per.tile([128, L], F16, tag="knat16")
        ksum = per.tile([128, NT * 128], F32, tag="ksum")    # l-tiles x d
        Mmax2 = per.tile([128, 32], F32, tag="Mmax2")
        Msum2 = per.tile([128, 32], F32, tag="Msum2")
        M2 = per.tile([128, 32], F32, tag="M2")
        iot = per.tile([128, 16], F32, tag="iot")

        for c in range(4):
            nc.sync.dma_start(out=wq_t[:, c * 128:(c + 1) * 128],
                              in_=wq[c * 128:(c + 1) * 128, :])
            nc.sync.dma_start(out=wk_t[:, c * 128:(c + 1) * 128],
                              in_=wk[c * 128:(c + 1) * 128, :])
            nc.sync.dma_start(out=wv_t[:, c * 128:(c + 1) * 128],
                              in_=wv[c * 128:(c + 1) * 128, :])
        nc.sync.dma_start(out=wo_t, in_=wo[:, :])
        nc.sync.dma_start(out=bq_t, in_=bq[:, :])
        nc.sync.dma_start(out=bk_t, in_=bk[:, :])
        nc.sync.dma_start(out=bv_t, in_=bv[:, :])
        nc.sync.dma_start(out=io_t, in_=io128[:, :])
        nc.sync.dma_start(out=tb_t, in_=tb64[:, :])
        for t in range(16):
            nc.vector.tensor_scalar(out=iot[:, t:t + 1], in0=io_t,
                                    scalar1=float(128 * t), scalar2=None,
                                    op0=ALU.add)

        ident = per.tile([128, 128], F32, tag="ident")
        from concourse.masks import make_identity
        make_identity(nc, ident)

        # ---- phase A: transposed loads of qb/kb/vb ----
        with tc.tile_pool(name="pA", bufs=8) as pa, \
             tc.tile_pool(name="pAp", bufs=2, space="PSUM") as pap:
            for (src, dstT) in ((qb, qbT), (kb, kbT), (vb, vbT)):
                for tg in range(4):
                    tiles = []
                    for i in range(4):
                        t = tg * 4 + i
                        xt = pa.tile([128, D], F32, tag="ld")
                        nc.sync.dma_start(out=xt,
                                          in_=src[t * 128:(t + 1) * 128, :])
                        tiles.append(xt)
                    for c in range(4):
                        ps = pap.tile([128, 512], F32, tag="tr")
                        for i in range(4):
                            nc.tensor.transpose(
                                ps[:, i * 128:(i + 1) * 128],
                                tiles[i][:, c * 128:(c + 1) * 128], ident)
                        nc.scalar.copy(out=dstT[c][:, tg * 512:(tg + 1) * 512],
                                       in_=ps)

        # ---- phase B: projections QT/KT/VT + V/Q/K natural ----
        with tc.tile_pool(name="pBp", bufs=2, space="PSUM") as pbp:
            for (srcT, dstP, w_t, b_t) in (
                (qbT, QT, wq_t, bq_t), (kbT, KT, wk_t, bk_t),
                (vbT, VT, wv_t, bv_t),
            ):
                for qc in range(4):
                    ps = pbp.tile([128, 512], F32, tag="proj")
                    for c in range(4):
                        nc.tensor.matmul(
                            ps, lhsT=w_t[:, c * 128:(c + 1) * 128],
                            rhs=srcT[c][:, qc * 512:(qc + 1) * 512],
                            start=(c == 0), stop=(c == 3))
                    nc.scalar.activation(
                        out=dstP[:, qc * 512:(qc + 1) * 512], in_=ps,
                        func=ACTF.Identity, bias=b_t, scale=1.0)
            # V natural + Q natural into vqnat; K natural fp16
            for tg in range(4):
                psv = pbp.tile([128, 512], F32, tag="trv")
                psq = pbp.tile([128, 512], F32, tag="trq")
                psk = pbp.tile([128, 512], F32, tag="trk")
                for i in range(4):
                    t = tg * 4 + i
                    nc.tensor.transpose(psv[:, i * 128:(i + 1) * 128],
                                        VT[:, t * 128:(t + 1) * 128], ident)
                    nc.tensor.transpose(psq[:, i * 128:(i + 1) * 128],
                                        QT[:, t * 128:(t + 1) * 128], ident)
                    nc.tensor.transpose(psk[:, i * 128:(i + 1) * 128],
                                        KT[:, t * 128:(t + 1) * 128], ident)
                for i in range(4):
                    t = tg * 4 + i
                    nc.scalar.copy(out=vqnat[:, t * 256:t * 256 + 128],
                                   in_=psv[:, i * 128:(i + 1) * 128])
                    nc.scalar.copy(out=vqnat[:, t * 256 + 128:t * 256 + 256],
                                   in_=psq[:, i * 128:(i + 1) * 128])
                    nc.scalar.copy(out=knat16[:, t * 128:(t + 1) * 128],
                                   in_=psk[:, i * 128:(i + 1) * 128])

        # ---- phase C: Ksum = counts @ K via PE, then Msum rowdots ----
        with tc.tile_pool(name="pC", bufs=2) as pc, \
             tc.tile_pool(name="pCp", bufs=1, space="PSUM") as pcp, \
             tc.tile_pool(name="pCp2", bufs=2, space="PSUM") as pcp2:
            ksT = pcp.tile([128, L], F32, tag="ksT")  # KsumT [d, l] 4 banks
            for jc in range(16):
                ct = pc.tile([128, L], FP8, tag="csT")
                nc.sync.dma_start(out=ct, in_=csumT[jc * 128:(jc + 1) * 128, :])
                for lc in range(4):
                    nc.tensor.matmul(
                        ksT[:, lc * 512:(lc + 1) * 512],
                        lhsT=knat16[:, jc * 128:(jc + 1) * 128],
                        rhs=ct[:, lc * 512:(lc + 1) * 512],
                        start=(jc == 0), stop=(jc == 15))
            ksT_sb = pc.tile([128, L], F32, tag="ksT_sb")
            for lc in range(4):
                nc.scalar.copy(out=ksT_sb[:, lc * 512:(lc + 1) * 512],
                               in_=ksT[:, lc * 512:(lc + 1) * 512])
            for tg in range(4):
                ps = pcp2.tile([128, 512], F32, tag="trks")
                for i in range(4):
                    t = tg * 4 + i
                    nc.tensor.transpose(ps[:, i * 128:(i + 1) * 128],
                                        ksT_sb[:, t * 128:(t + 1) * 128], ident)
                nc.scalar.copy(out=ksum[:, tg * 512:(tg + 1) * 512], in_=ps)
            junk64 = pc.tile([128, 64], F32, tag="junk64")
            for t in range(NT):
                for h in range(2):
                    nc.vector.scalar_tensor_tensor(
                        out=junk64,
                        in0=vqnat[:, t * 256 + 128 + 64 * h: t * 256 + 128 + 64 * (h + 1)],
                        scalar=0.0,
                        in1=ksum[:, t * 128 + 64 * h: t * 128 + 64 * (h + 1)],
                        op0=ALU.bypass, op1=ALU.mult,
                        accum_out=Msum2[:, h * 16 + t: h * 16 + t + 1])

        # ---- phase D: S, mask folded into PSUM via fp8 identity matmul,
        #      then plain max-reduce ----
        ident8 = per.tile([128, 128], FP8, tag="ident8")
        nc.scalar.copy(out=ident8, in_=ident)
        with tc.tile_pool(name="pD", bufs=2) as pd, \
             tc.tile_pool(name="pDp", bufs=2, space="PSUM") as pdp:
            for t in range(NT):
                cm = pd.tile([128, L], FP8, tag="cm")
                nc.sync.dma_start(out=cm, in_=cmax[t * 128:(t + 1) * 128, :])
                for h in range(2):
                    sp = pdp.tile([128, L], F32, tag="S")
                    for jc in range(4):
                        nc.tensor.matmul(
                            sp[:, jc * 512:(jc + 1) * 512],
                            lhsT=QT[64 * h:64 * (h + 1), t * 128:(t + 1) * 128],
                            rhs=KT[64 * h:64 * (h + 1), jc * 512:(jc + 1) * 512],
                            start=True, stop=False)
                        nc.tensor.matmul(
                            sp[:, jc * 512:(jc + 1) * 512],
                            lhsT=ident8,
                            rhs=cm[:, jc * 512:(jc + 1) * 512],
                            start=False, stop=True)
                    nc.vector.tensor_reduce(
                        out=Mmax2[:, h * 16 + t: h * 16 + t + 1], in_=sp,
                        axis=mybir.AxisListType.X, op=ALU.max)

        # ---- phase E: M, exact ordered global top-40 per head ----
        with tc.tile_pool(name="pE", bufs=1) as pe, \
             tc.tile_pool(name="pEp", bufs=2, space="PSUM") as pep:
            nc.vector.scalar_tensor_tensor(
                out=M2, in0=Msum2, scalar=-1.0 / L, in1=Mmax2,
                op0=ALU.mult, op1=ALU.add)
            mt32 = pe.tile([64, 128], F32, tag="mt32")
            nc.vector.memset(mt32, NEG)
            for h in range(2):
                ps = pep.tile([16, 128], F32, tag="mtr")
                nc.tensor.transpose(ps, M2[:, h * 16:(h + 1) * 16], ident)
                nc.scalar.copy(out=mt32[32 * h:32 * h + 16, :], in_=ps)
            # level 1: per-partition (= per row-tile) ordered top-40
            cval = pe.tile([64, 48], F32, tag="cval")
            cidx = pe.tile([64, 48], U32, tag="cidx")
            cidxf = pe.tile([64, 48], F32, tag="cidxf")
            wa = pe.tile([64, 128], F32, tag="wa")
            wb = pe.tile([64, 128], F32, tag="wb")
            nc.vector.memset(cval, 0.0)
            nc.vector.memset(cidx, 0)
            nc.vector.tensor_copy(out=wa, in_=mt32)
            cur, nxt = wa, wb
            for r in range(5):
                nc.vector.max(out=cval[:, r * 8:(r + 1) * 8], in_=cur)
                nc.vector.max_index(out=cidx[:, r * 8:(r + 1) * 8],
                                    in_max=cval[:, r * 8:(r + 1) * 8],
                                    in_values=cur)
                if r < 4:
                    nc.vector.match_replace(
                        out=nxt, in_to_replace=cval[:, r * 8:(r + 1) * 8],
                        in_values=cur, imm_value=NEG)
                    cur, nxt = nxt, cur
            nc.vector.tensor_copy(out=cidxf, in_=cidx)
            nc.vector.tensor_scalar(out=cidxf, in0=cidxf, scalar1=tb_t,
                                    scalar2=None, op0=ALU.add)
            # flatten candidates into [32, 640] merge arrays
            w640a = pe.tile([32, 640], F32, tag="w640a")
            w640b = pe.tile([32, 640], F32, tag="w640b")
            d640 = pe.tile([128, 640], F32, tag="d640")
            nc.vector.memset(w640a, NEG)
            nc.vector.memset(w640b, NEG)
            nc.vector.memset(d640, 0.0)
            for h in range(2):
                nc.sync.dma_start(out=w640a[16 * h:16 * h + 1, :],
                                  in_=cval[h * 32:h * 32 + 16, 0:40])
                nc.sync.dma_start(out=d640[16 * h:16 * h + 1, :],
                                  in_=cidxf[h * 32:h * 32 + 16, 0:40])
            # level 2: merge 640 candidates -> global ordered top-40 positions
            mx8 = pe.tile([32, 8], F32, tag="mx8")
            pos = pe.tile([32, 48], U32, tag="pos")
            pos16 = pe.tile([128, 48], U16, tag="pos16")
            nc.vector.memset(pos, 0)
            nc.vector.memset(pos16, 0)
            cur, nxt = w640a, w640b
            for r in range(5):
                nc.vector.max(out=mx8, in_=cur)
                nc.vector.max_index(out=pos[:, r * 8:(r + 1) * 8],
                                    in_max=mx8, in_values=cur)
                if r < 4:
                    nc.vector.match_replace(out=nxt, in_to_replace=mx8,
                                            in_values=cur, imm_value=NEG)
                    cur, nxt = nxt, cur
            nc.vector.tensor_copy(out=pos16[0:32, :], in_=pos)
            # wrap positions into 16-partition groups for indirect_copy
            idxw = pe.tile([128, 3], U16, tag="idxw")
            nc.vector.memset(idxw, 0)
            for h in range(2):
                for s in range(3):
                    nc.sync.dma_start(
                        out=idxw[16 * h:16 * (h + 1), s:s + 1],
                        in_=pos16[16 * h:16 * h + 1, s * 16:(s + 1) * 16])
            top128 = pe.tile([128, 40], F32, tag="top128")
            nc.gpsimd.indirect_copy(out=top128, data=d640, idxs=idxw,
                                    i_know_ap_gather_is_preferred=True)
            # broadcast each head's ordered top-40 row to all partitions
            ones1 = pe.tile([1, 128], F32, tag="ones1")
            nc.vector.memset(ones1, 1.0)
            topbc = pe.tile([128, 80], F32, tag="topbc")
            t40_0 = pe.tile([1, 40], F32, tag="t40_0")
            t40_1 = pe.tile([1, 40], F32, tag="t40_1")
            for h, t40 in ((0, t40_0), (1, t40_1)):
                nc.sync.dma_start(out=t40,
                                  in_=top128[16 * h:16 * h + 1, :])
            for h, t40 in ((0, t40_0), (1, t40_1)):
                ps = pep.tile([128, 40], F32, tag="tbc")
                nc.tensor.matmul(ps, lhsT=ones1, rhs=t40,
                                 start=True, stop=True)
                nc.scalar.copy(out=topbc[:, 40 * h:40 * (h + 1)], in_=ps)

        # ---- phase F: one-hot E matmuls; gather V/Q rows + Vsum ----
        with tc.tile_pool(name="pF", bufs=2) as pf, \
             tc.tile_pool(name="pFp", bufs=1, space="PSUM") as pfp, \
             tc.tile_pool(name="pG", bufs=1) as pg, \
             tc.tile_pool(name="pGp", bufs=1, space="PSUM") as pgp:
            eps = [pfp.tile([41, 256], F32, tag=f"eps{h}", name=f"eps{h}") for h in range(2)]
            for t in range(NT):
                for h in range(2):
                    et = pf.tile([128, 41], F32, tag="et")
                    nc.vector.tensor_scalar(
                        out=et[:, 0:40], in0=topbc[:, 40 * h:40 * (h + 1)],
                        scalar1=iot[:, t:t + 1], scalar2=None, op0=ALU.is_equal)
                    nc.vector.memset(et[:, 40:41], 1.0)
                    nc.tensor.matmul(eps[h], lhsT=et,
                                     rhs=vqnat[:, t * 256:(t + 1) * 256],
                                     start=(t == 0), stop=(t == 15))
            # ---- phase G: tail context math ----
            ctx2 = pg.tile([41, 128], F32, tag="ctx2")
            for h in range(2):
                esb = pg.tile([41, 256], F32, tag=f"esb{h}")
                nc.scalar.copy(out=esb, in_=eps[h])
                vs = slice(64 * h, 64 * (h + 1))
                qs = slice(128 + 64 * h, 128 + 64 * (h + 1))
                # x = Qsel . K[:, L-1]
                qselT_ps = pgp.tile([64, 40], F32, tag="qselT")
                nc.tensor.transpose(qselT_ps, esb[0:40, qs], ident[0:40, 0:40])
                qselT = pg.tile([64, 40], F32, tag=f"qselTs{h}")
                nc.scalar.copy(out=qselT, in_=qselT_ps)
                k2047 = pg.tile([64, 1], F32, tag=f"k2047_{h}",
                                name=f"k2047_{h}")
                nc.sync.dma_start(out=k2047,
                                  in_=KT[64 * h:64 * (h + 1), L - 1:L])
                xps = pgp.tile([40, 1], F32, tag="xps")
                nc.tensor.matmul(xps, lhsT=qselT, rhs=k2047,
                                 start=True, stop=True)
                e_t = pg.tile([40, 1], F32, tag=f"e{h}")
                nc.scalar.activation(out=e_t, in_=xps, func=ACTF.Exp)
                em1 = pg.tile([40, 1], F32, tag=f"em1{h}")
                den = pg.tile([40, 1], F32, tag=f"den{h}")
                rec = pg.tile([40, 1], F32, tag=f"rec{h}")
                nc.vector.tensor_scalar(out=em1, in0=e_t, scalar1=-1.0,
                                        scalar2=None, op0=ALU.add)
                nc.vector.tensor_scalar(out=den, in0=e_t, scalar1=float(L - 1),
                                        scalar2=None, op0=ALU.add)
                nc.vector.reciprocal(out=rec, in_=den)
                # Vsum broadcast over the 40 rows
                ones40 = pg.tile([1, 40], F32, tag="ones40")
                nc.vector.memset(ones40, 1.0)
                vsrow = pg.tile([1, 64], F32, tag=f"vsrow{h}", name=f"vsrow{h}")
                nc.sync.dma_start(out=vsrow, in_=esb[40:41, vs])
                vsb_ps = pgp.tile([40, 64], F32, tag="vsb")
                nc.tensor.matmul(vsb_ps, lhsT=ones40, rhs=vsrow,
                                 start=True, stop=True)
                tmp = pg.tile([40, 64], F32, tag=f"tmp{h}")
                nc.vector.tensor_scalar(out=tmp, in0=esb[0:40, vs],
                                        scalar1=em1, scalar2=None, op0=ALU.mult)
                tmp2 = pg.tile([40, 64], F32, tag=f"tmp2{h}")
                nc.vector.tensor_add(tmp2, tmp, vsb_ps)
                nc.vector.tensor_scalar(out=ctx2[0:40, vs], in0=tmp2,
                                        scalar1=rec, scalar2=None, op0=ALU.mult)
                vcom = pg.tile([1, 64], F32, tag=f"vcom{h}", name=f"vcom{h}")
                nc.scalar.mul(out=vcom, in_=vsrow, mul=1.0 / L)
                nc.sync.dma_start(out=ctx2[40:41, vs], in_=vcom)
            # ---- final projection ----
            c2T_ps = pgp.tile([128, 41], F32, tag="c2T")
            nc.tensor.transpose(c2T_ps, ctx2, ident[0:41, 0:41])
            c2T = pg.tile([128, 41], F32, tag="c2Ts")
            nc.scalar.copy(out=c2T, in_=c2T_ps)
            ops = pgp.tile([41, D], F32, tag="ops")
            nc.tensor.matmul(ops, lhsT=c2T, rhs=wo_t, start=True, stop=True)
            osb = pg.tile([41, D], F32, tag="osb")
            nc.scalar.copy(out=osb, in_=ops)
            nc.sync.dma_start(out=outp[:, :], in_=osb)

        per.release()
    return nc


_NC = None


def _get_nc():
    global _NC
    if _NC is None:
        _NC = build_nc()
    return _NC


def _host_prep(q, k, v, Wq, bq, Wk, bk, Wv, bv, Wo, bo, index_sample):
    """Build the 8 per-core input maps."""
    idx = np.asarray(index_sample).astype(np.int64)
    flat = (np.arange(L, dtype=np.int64)[:, None] * L + idx).ravel()
    counts = np.bincount(flat, minlength=L * L).reshape(L, L)
    cmax = np.where(counts > 0, 0.0, -240.0).astype(ml_dtypes.float8_e4m3)
    csumT = np.ascontiguousarray(counts.T).astype(ml_dtypes.float8_e4m3)
    io128 = np.arange(128, dtype=np.float32).reshape(128, 1)
    tb64 = (128.0 * (np.arange(64) % 32)).astype(np.float32).reshape(64, 1)

    f32 = np.float32
    q, k, v = (np.asarray(x, f32) for x in (q, k, v))
    Wq, Wk, Wv, Wo = (np.ascontiguousarray(np.asarray(x, f32)) for x in (Wq, Wk, Wv, Wo))
    bq, bk, bv = (np.asarray(x, f32) for x in (bq, bk, bv))

    maps = []
    for c in range(8):
        b = c // 4
        hh = c % 4
        cs = slice(128 * hh, 128 * (hh + 1))
        maps.append({
            "qb": np.ascontiguousarray(q[b]),
            "kb": np.ascontiguousarray(k[b]),
            "vb": np.ascontiguousarray(v[b]),
            "wq": np.ascontiguousarray(Wq[:, cs]),
            "wk": np.ascontiguousarray(Wk[:, cs]),
            "wv": np.ascontiguousarray(Wv[:, cs]),
            "wo": np.ascontiguousarray(Wo[cs, :]),
            "bq": np.ascontiguousarray(bq[cs].reshape(128, 1)),
            "bk": np.ascontiguousarray(bk[cs].reshape(128, 1)),
            "bv": np.ascontiguousarray(bv[cs].reshape(128, 1)),
            "cmax": cmax,
            "csumT": csumT,
            "io128": io128,
            "tb64": tb64,
        })
    return maps


def kernel(**inputs):
    nc = _get_nc()
    maps = _host_prep(**inputs)
    res = run_bass_kernel_spmd(nc, maps, core_ids=list(range(8)))
    bo = np.asarray(inputs["bo"], np.float32)
    out = np.empty((B, L, D), np.float32)
    for b in range(2):
        part = sum(res.results[4 * b + i]["outp"] for i in range(4))
        part = part + bo[None, :]
        out[b, 0:NTOP, :] = part[0:NTOP]
        out[b, NTOP:, :] = part[NTOP]
    return out
